# revision 18
# baseline (speedup 1.0000x reference)
"""Trainium2 Bass kernel for nn_Attention (cross-attention, B=2 S=2048 D=1024 H=16).

Sharding: 8 cores = data-parallel over batch (2) x tensor-parallel over head
groups (4 groups of 4 heads). Each core computes q/k/v projections for its
256 output dims plus softmax(QK^T)V for its 4 heads; outputs are disjoint
slices of the full output, gathered host-side (no collectives).

Layout strategy (all matmul operands bf16, PSUM fp32):
  qT/kT [dim, token]   <- W^T stationary, x^T streamed (x^T built host-side)
  scoresT[j, i]        <- kT chunk stationary (K=64), qT streamed. The two
    heads of a pair run CONCURRENTLY on PE row-tiles T0/T8 (64x128 mode) and
    write the two halves of one [128, 2, 512] PSUM tile (different banks, as
    row tiles require), so both banks are granted/freed together.
  exp straight out of PSUM, one [128, 1024] op per score pair, split between
    ScalarE (table exp, per-op overhead amortized over 1024 elements) and DVE
    (Schraudolph bit-trick: bf16 bits = round(x*128/ln2 + 16252), one
    tensor_scalar). GpSimd cannot access PSUM on TRN2 so it only does
    memsets. Softmax max-subtraction dropped: |scores| < ~6 here.
  out[i, c] accum      <- et[j,i] chunk stationary, [v | ones] streamed (N=65
    matmuls issue back-to-back at ~30ns). The ones column lands the softmax
    denominator at free-column 64, i.e. a per-partition scalar: reciprocal is
    a [128,1] op and the normalize is a Copy-activation with per-partition
    scale AP. Each 128-i block accumulates in its own PSUM bank (start=True
    in a shared bank would zero sibling regions - HW zeroes the whole bank).
Emission is software-pipelined: the AV groups of sub-block n-1 are spliced
into the score stream of sub-block n (one 16-matmul group per 2 score pairs),
so the PE stays busy while the exp engines drain PSUM; the attention phase is
exp-engine-bound, so the ~95ns PE mode switches at splice points are hidden.
"""

import numpy as np
import ml_dtypes

import concourse.bass as bass
import concourse.mybir as mybir
import concourse.tile as tile
from concourse.bass_utils import run_bass_kernel_spmd

B, S, D, H = 2, 2048, 1024, 16
HD = D // H  # 64 head dim
N_CORES = 8
HG = 4  # head groups = cores per batch entry
DH = D // HG  # 256 output dims per core
HPC = H // HG  # 4 heads per core
NF = D // 128  # 8 feature (contraction) chunks
F32 = mybir.dt.float32
BF16 = mybir.dt.bfloat16
I16 = mybir.dt.int16
EXP = mybir.ActivationFunctionType.Exp
MULT = mybir.AluOpType.mult
ADD = mybir.AluOpType.add

# bf16 fast-exp: bits16 = round(x * 128/ln2 + (127*128 - C)); C centers the
# multiplicative sawtooth error at ~+-4.2% (measured 4.15% max on HW).
A_EXP = float(128.0 / np.log(2.0))
B_EXP = 16252.0


def _split_excess_waits(nc, cap=1):
    """This container's walrus caps sync waits at 1/instruction. Hoist excess
    waits onto InstNoOps inserted just before the instruction (same engine)."""
    ctr = 0
    spread = [
        mybir.EngineType.SP,
        mybir.EngineType.Pool,
        mybir.EngineType.PE,
        mybir.EngineType.DVE,
        mybir.EngineType.Activation,
    ]
    for bb in nc.main_func.blocks:
        insts = list(bb.instructions)
        out = []
        changed = False
        for inst in insts:
            si = inst.sync_info
            waits = list(si.on_wait) if (si is not None and si.on_wait) else []
            if len(waits) > cap:
                changed = True
                # the tail drain carries ~25 waits; spreading its wait NoOps
                # across engines lets them wait in parallel (the barrier that
                # follows gathers every engine anyway)
                is_tail = type(inst).__name__ == "InstDrain" and len(waits) > 6
                for i, w in enumerate(waits[:-cap]):
                    ctr += 1
                    eng = spread[i % len(spread)] if is_tail else inst.engine
                    out.append(
                        mybir.InstNoOp(
                            name=f"I-waitsplit-{ctr}",
                            sync_info=mybir.SyncInfo(on_wait=[w], on_update=[]),
                            engine=eng,
                            ins=[],
                            outs=[],
                        )
                    )
                inst.sync_info = mybir.SyncInfo(
                    on_wait=waits[-cap:], on_update=list(si.on_update or [])
                )
            out.append(inst)
        if changed:
            bb.instructions = out
    return ctr


def build_nc(s=S, split_waits=True):
    """One core's program (SPMD: all cores run it on their own shard)."""
    nj = s // 128  # j (key token) chunks
    TOK = 1024  # projection token tile
    ntt = s // TOK
    IC = 512  # attention i sub-block width
    nic = s // IC
    nib = IC // 128  # 128-i-blocks per sub-block

    nc = bass.Bass()
    xT = nc.dram_tensor("xT", [D, s], BF16, kind="ExternalInput")
    cT = nc.dram_tensor("cT", [D, s], BF16, kind="ExternalInput")
    wall = nc.dram_tensor("wall", [3 * D, DH], BF16, kind="ExternalInput")
    out = nc.dram_tensor("out", [s, DH], F32, kind="ExternalOutput")

    with tile.TileContext(nc) as tc:
        with (
            tc.tile_pool(name="w", bufs=1) as wpool,
            tc.tile_pool(name="stream", bufs=4) as spool,
            tc.tile_pool(name="qk", bufs=1) as qkpool,
            tc.tile_pool(name="vab", bufs=nj) as vpool,
            tc.tile_pool(name="et", bufs=33) as epool,
            tc.tile_pool(name="fin", bufs=8) as fpool,
            tc.tile_pool(name="ob", bufs=4) as opool,
        ):
            # resident weights [feat_part, tensor, feat_chunk, outdim]
            w_all = wpool.tile([128, 3, NF, DH], BF16, tag="wall")

            def load_w(t, split=1):
                wr = wall[t * D : (t + 1) * D].rearrange("(f p) o -> p f o", p=128)
                fh = NF // split
                for k in range(split):
                    nc.sync.dma_start(
                        w_all[:, t, k * fh : (k + 1) * fh], wr[:, k * fh : (k + 1) * fh]
                    )

            load_w(0, split=2)

            xTr = xT.rearrange("(f p) t -> p f t", p=128)
            cTr = cT.rearrange("(f p) t -> p f t", p=128)

            ENGS = [nc.scalar, nc.vector]
            ectr = [0]

            def next_eng():
                e = ENGS[ectr[0] % len(ENGS)]
                ectr[0] += 1
                return e

            def emit_exp(eng, et_ap, psc_ap):
                if eng is nc.scalar:
                    eng.activation(et_ap, psc_ap, EXP)
                else:
                    eng.tensor_scalar(
                        et_ap.bitcast(I16), psc_ap, A_EXP, B_EXP, MULT, ADD
                    )

            def emit_copy(eng, dst, src):
                if eng is nc.scalar:
                    eng.copy(dst, src)
                else:
                    eng.tensor_copy(dst, src)

            def emit_scale(eng, dst, src, scale_ap):
                if eng is nc.scalar:
                    eng.mul(dst, src, scale_ap)
                else:
                    eng.tensor_scalar(dst, src, scale_ap, None, MULT)

            qT = [
                qkpool.tile([128, s], BF16, tag="qT0", name="q_o0"),
                qkpool.tile([128, s], BF16, tag="qT1", name="q_o1"),
            ]
            kT = [
                qkpool.tile([128, s], BF16, tag="kT0", name="k_o0"),
                qkpool.tile([128, s], BF16, tag="kT1", name="k_o1"),
            ]
            vab = [None] * nj

            # ---- projection phase (128x128 mode) ----
            with tc.tile_pool(name="pj", bufs=2, space="PSUM") as pj:

                def load_tok(src_r, i, nm):
                    # two half-loads split along TOKENS (each 512-token proj
                    # chunk accumulates over all 8 f, so an f-split would gate
                    # the first chunk on the full tile)
                    t = spool.tile([128, NF, TOK], BF16, tag="st", name=nm)
                    half = TOK // 2
                    tr = src_r[:, :, i * TOK : (i + 1) * TOK]
                    nc.sync.dma_start(t[:, :, :half], tr[:, :, :half])
                    nc.sync.dma_start(t[:, :, half:], tr[:, :, half:])
                    return t

                def proj_chunk(w_sb, tok, o, dst_sl):
                    pq = pj.tile([128, TOK], F32, tag="pp", name="pq")
                    for w0 in range(0, TOK, 512):
                        for f in range(NF):
                            nc.tensor.matmul(
                                pq[:, w0 : w0 + 512],
                                w_sb[:, f, o * 128 : (o + 1) * 128],
                                tok[:, f, w0 : w0 + 512],
                                start=(f == 0),
                                stop=(f == NF - 1),
                            )
                    emit_copy(next_eng(), dst_sl, pq[:])

                # prologue: only the o=0 q/k chunks (which gate the first
                # attention sub-blocks) plus all of V (which gates the first
                # spliced AV group). The o=1 chunks are deferred into the
                # attention stream as PE filler.
                tpj = TOK // 128
                xt, ct = [], []
                for ib in range(ntt):
                    xt.append(load_tok(xTr, ib, f"xt{ib}"))
                    if ib == 0:
                        load_w(1)
                    ct.append(load_tok(cTr, ib, f"ct{ib}"))
                    if ib == 0:
                        load_w(2)
                    sl = slice(ib * TOK, (ib + 1) * TOK)
                    proj_chunk(w_all[:, 0], xt[ib], 0, qT[0][:, sl])
                    proj_chunk(w_all[:, 1], ct[ib], 0, kT[0][:, sl])
                    for jc in range(ib * tpj, (ib + 1) * tpj):
                        # v[j, o] = sum_f cT[f,j] * WvT[f,o]
                        pvv = pj.tile([128, TOK], F32, tag="pp", name="pvv")
                        for f in range(NF):
                            nc.tensor.matmul(
                                pvv[:, :DH],
                                ct[jc // tpj][:, f, (jc % tpj) * 128 : (jc % tpj + 1) * 128],
                                w_all[:, 2, f, :],
                                start=(f == 0),
                                stop=(f == NF - 1),
                            )
                        va = vpool.tile([128, HPC, HD + 1], BF16, tag="vab", name="va")
                        emit_copy(
                            next_eng(),
                            va[:, :, :HD],
                            pvv[:, :DH].rearrange("p (h c) -> p h c", c=HD),
                        )
                        nc.gpsimd.memset(va[:, :, HD], 1.0)
                        vab[jc] = va

            # ---- attention phase ----
            # sub-block = (head pair hp, i chunk ic). Scores+exp of sub-block n
            # are emitted with the AV groups of sub-block n-1 spliced in (one
            # 16-matmul group per 2 score pairs).
            with (
                tc.tile_pool(name="ps", bufs=3, space="PSUM") as ps,
                tc.tile_pool(name="pv", bufs=2, space="PSUM") as pv,
            ):

                def av_group(hp, ic, ets, h01, ib):
                    h = hp * 2 + h01
                    ppv = pv.tile([128, HD + 1], F32, tag="pv", name="ppv")
                    for jt in range(nj):
                        nc.tensor.matmul(
                            ppv[:],
                            ets[jt][:, h01, ib * 128 : (ib + 1) * 128],
                            vab[jt][:, h, :],
                            start=(jt == 0),
                            stop=(jt == nj - 1),
                        )
                    # finalize: per-partition denom at free col HD
                    rd = fpool.tile([128, 1], F32, tag="rd", name="rd")
                    nc.vector.reciprocal(rd[:], ppv[:, HD : HD + 1])
                    ob = obs[h01]
                    emit_scale(
                        next_eng(),
                        ob[:, ib * HD : (ib + 1) * HD],
                        ppv[:, :HD],
                        rd[:],
                    )
                    if ib == nib - 1:
                        nc.sync.dma_start(
                            out[ic * IC : (ic + 1) * IC, h * HD : (h + 1) * HD]
                            .rearrange("(b p) c -> p b c", p=128),
                            ob[:].rearrange("p (b c) -> p b c", c=HD),
                        )

                subs = [(hp, ic) for hp in range(2) for ic in range(nic)]
                pend = None  # (hp, ic, ets) awaiting AV emission
                obs = None
                # exp engine per score pair: ScalarE amortizes per-op overhead
                # better, so it takes 9 of 16; DVE also does recips + half the
                # normalize muls.
                act_jts = {0, 2, 4, 6, 7, 9, 11, 13, 15}

                def proj_fill(t, o, ib):
                    # deferred o=1 projection chunk as attention-phase PE
                    # filler; borrows a score-pool PSUM tile (same bank shape)
                    pq = ps.tile([128, 2, IC], F32, tag="sc", name="pqf")
                    tok = (xt if t == 0 else ct)[ib]
                    for half in range(2):
                        for f in range(NF):
                            nc.tensor.matmul(
                                pq[:, half],
                                w_all[:, t, f, o * 128 : (o + 1) * 128],
                                tok[:, f, half * IC : (half + 1) * IC],
                                start=(f == 0),
                                stop=(f == NF - 1),
                            )
                    dst = (qT if t == 0 else kT)[o]
                    emit_copy(
                        next_eng(),
                        dst[:, ib * TOK : (ib + 1) * TOK],
                        pq[:].rearrange("p a b -> p (a b)"),
                    )

                fillers = [(1, 1, 0), (1, 1, 1), (0, 1, 0), (0, 1, 1)]

                for n, (hp, ic) in enumerate(subs):
                    ets = [None] * nj
                    if pend is not None:
                        pgroups = [(h01, ib) for h01 in range(2) for ib in range(nib)]
                        obs = [
                            opool.tile([128, nib * HD], F32, tag="ob", name="ob0"),
                            opool.tile([128, nib * HD], F32, tag="ob", name="ob1"),
                        ]
                    for jt in range(nj):
                        # both heads of the pair concurrently on row-tiles
                        # T0/T8, into the two banks of one PSUM tile
                        psc = ps.tile([128, 2, IC], F32, tag="sc", name="psc")
                        for h01 in range(2):
                            pb = h01 * 64
                            nc.tensor.matmul(
                                psc[:, h01],
                                kT[hp][pb : pb + 64, jt * 128 : (jt + 1) * 128],
                                qT[hp][pb : pb + 64, ic * IC : (ic + 1) * IC],
                                start=True,
                                stop=True,
                            )
                        et = epool.tile([128, 2, IC], BF16, tag="et", name="et")
                        eng = nc.scalar if jt in act_jts else nc.vector
                        emit_exp(eng, et[:], psc[:])
                        ets[jt] = et
                        if pend is not None and jt % 2 == 1:
                            av_group(pend[0], pend[1], pend[2], *pgroups[jt // 2])
                        if n < len(fillers) and jt == 7:
                            proj_fill(*fillers[n])
                    pend = (hp, ic, ets)

                # drain: AV of the last sub-block
                obs = [
                    opool.tile([128, nib * HD], F32, tag="ob", name="ob0"),
                    opool.tile([128, nib * HD], F32, tag="ob", name="ob1"),
                ]
                for h01 in range(2):
                    for ib in range(nib):
                        av_group(pend[0], pend[1], pend[2], h01, ib)

    if split_waits:
        _split_excess_waits(nc)
    return nc


def make_in_maps(x, context, Wq, Wkv, s=S):
    """Host-side shard + layout prep. Core c -> (batch c//HG, head group c%HG)."""
    x = np.asarray(x, dtype=np.float32)
    context = np.asarray(context, dtype=np.float32)
    Wq = np.asarray(Wq, dtype=np.float32)
    Wkv = np.asarray(Wkv, dtype=np.float32)
    scale = np.float32(HD**-0.5)
    bf16 = ml_dtypes.bfloat16
    xTb = [np.ascontiguousarray(x[b].T).astype(bf16) for b in range(B)]
    cTb = [np.ascontiguousarray(context[b].T).astype(bf16) for b in range(B)]
    in_maps = []
    for core in range(N_CORES):
        b, hg = core // HG, core % HG
        sl = slice(hg * DH, (hg + 1) * DH)
        in_maps.append(
            {
                "xT": xTb[b],
                "cT": cTb[b],
                "wall": np.ascontiguousarray(
                    np.concatenate(
                        [
                            Wq[sl].T * scale,
                            Wkv[sl].T,
                            Wkv[D + hg * DH : D + (hg + 1) * DH].T,
                        ],
                        axis=0,
                    )
                ).astype(bf16),
            }
        )
    return in_maps


def gather_out(results, s=S):
    full = np.empty((B, s, D), dtype=np.float32)
    for core in range(N_CORES):
        b, hg = core // HG, core % HG
        full[b, :, hg * DH : (hg + 1) * DH] = results[core]["out"]
    return full


def kernel(x, context, Wq, Wkv):
    nc = build_nc(S)
    in_maps = make_in_maps(x, context, Wq, Wkv, S)
    res = run_bass_kernel_spmd(nc, in_maps, list(range(N_CORES)))
    return gather_out(res.results, S)


# revision 21
# speedup vs baseline: 1.1937x; 1.1937x over previous
"""Trainium2 Bass kernel for nn_Attention (cross-attention, B=2 S=2048 D=1024 H=16).

Sharding: 8 cores = data-parallel over batch (2) x tensor-parallel over head
groups (4 groups of 4 heads). Each core computes q/k/v projections for its
256 output dims plus softmax(QK^T)V for its 4 heads; outputs are disjoint
slices of the full output, gathered host-side (no collectives).

Layout strategy (all matmul operands bf16, PSUM fp32):
  qT/kT [dim, token]   <- W^T stationary, x^T streamed (x^T built host-side)
  scoresT[j, i]        <- kT chunk stationary (K=64), qT streamed. The two
    heads of a pair run CONCURRENTLY on PE row-tiles T0/T8 (64x128 mode) and
    write the two halves of one [128, 2, 512] PSUM tile (different banks, as
    row tiles require), so both banks are granted/freed together.
  exp straight out of PSUM, one [128, 1024] op per score pair, split between
    ScalarE (table exp, per-op overhead amortized over 1024 elements) and DVE
    (Schraudolph bit-trick: bf16 bits = round(x*128/ln2 + 16252), one
    tensor_scalar). GpSimd cannot access PSUM on TRN2 so it only does
    memsets. Softmax max-subtraction dropped: |scores| < ~6 here.
  out[i, c] accum      <- et[j,i] chunk stationary, [v | ones] streamed (N=65
    matmuls issue back-to-back at ~30ns). The ones column lands the softmax
    denominator at free-column 64, i.e. a per-partition scalar: reciprocal is
    a [128,1] op and the normalize is a Copy-activation with per-partition
    scale AP. Each 128-i block accumulates in its own PSUM bank (start=True
    in a shared bank would zero sibling regions - HW zeroes the whole bank).
Emission is software-pipelined: the AV groups of sub-block n-1 are spliced
into the score stream of sub-block n (one 16-matmul group per 2 score pairs),
so the PE stays busy while the exp engines drain PSUM; the attention phase is
exp-engine-bound, so the ~95ns PE mode switches at splice points are hidden.
"""

import numpy as np
import ml_dtypes

import concourse.bass as bass
import concourse.mybir as mybir
import concourse.tile as tile
from concourse.bass_utils import run_bass_kernel_spmd

B, S, D, H = 2, 2048, 1024, 16
HD = D // H  # 64 head dim
N_CORES = 8
HG = 4  # head groups = cores per batch entry
DH = D // HG  # 256 output dims per core
HPC = H // HG  # 4 heads per core
NF = D // 128  # 8 feature (contraction) chunks
F32 = mybir.dt.float32
BF16 = mybir.dt.bfloat16
I16 = mybir.dt.int16
EXP = mybir.ActivationFunctionType.Exp
MULT = mybir.AluOpType.mult
ADD = mybir.AluOpType.add

# bf16 fast-exp: bits16 = round(x * 128/ln2 + (127*128 - C)); C centers the
# multiplicative sawtooth error at ~+-4.2% (measured 4.15% max on HW).
A_EXP = float(128.0 / np.log(2.0))
B_EXP = 16252.0


def _split_excess_waits(nc, cap=1):
    """This container's walrus caps sync waits at 1/instruction. Hoist excess
    waits onto InstNoOps inserted just before the instruction (same engine)."""
    ctr = 0
    spread = [
        mybir.EngineType.SP,
        mybir.EngineType.Pool,
        mybir.EngineType.PE,
        mybir.EngineType.DVE,
        mybir.EngineType.Activation,
    ]
    for bb in nc.main_func.blocks:
        insts = list(bb.instructions)
        out = []
        changed = False
        for inst in insts:
            si = inst.sync_info
            waits = list(si.on_wait) if (si is not None and si.on_wait) else []
            if len(waits) > cap:
                changed = True
                # the tail drain carries ~25 waits; spreading its wait NoOps
                # across engines lets them wait in parallel (the barrier that
                # follows gathers every engine anyway)
                is_tail = type(inst).__name__ == "InstDrain" and len(waits) > 6
                for i, w in enumerate(waits[:-cap]):
                    ctr += 1
                    eng = spread[i % len(spread)] if is_tail else inst.engine
                    out.append(
                        mybir.InstNoOp(
                            name=f"I-waitsplit-{ctr}",
                            sync_info=mybir.SyncInfo(on_wait=[w], on_update=[]),
                            engine=eng,
                            ins=[],
                            outs=[],
                        )
                    )
                inst.sync_info = mybir.SyncInfo(
                    on_wait=waits[-cap:], on_update=list(si.on_update or [])
                )
            out.append(inst)
        if changed:
            bb.instructions = out
    return ctr


def build_nc(s=S, split_waits=True):
    """One core's program (SPMD: all cores run it on their own shard)."""
    nj = s // 128  # j (key token) chunks
    TOK = 1024  # projection token tile
    ntt = s // TOK
    IC = 512  # attention i sub-block width
    nic = s // IC
    nib = IC // 128  # 128-i-blocks per sub-block

    nc = bass.Bass()
    xT = nc.dram_tensor("xT", [D, s], BF16, kind="ExternalInput")
    cT = nc.dram_tensor("cT", [D, s], BF16, kind="ExternalInput")
    wall = nc.dram_tensor("wall", [3 * D, DH], BF16, kind="ExternalInput")
    out = nc.dram_tensor("out", [s, DH], F32, kind="ExternalOutput")

    with tile.TileContext(nc) as tc:
        with (
            tc.tile_pool(name="w", bufs=1) as wpool,
            tc.tile_pool(name="stream", bufs=4) as spool,
            tc.tile_pool(name="qk", bufs=1) as qkpool,
            tc.tile_pool(name="vab", bufs=nj) as vpool,
            tc.tile_pool(name="et", bufs=33) as epool,
            tc.tile_pool(name="fin", bufs=8) as fpool,
            tc.tile_pool(name="ob", bufs=4) as opool,
        ):
            # resident weights [feat_part, tensor, feat_chunk, outdim]
            w_all = wpool.tile([128, 3, NF, DH], BF16, tag="wall")

            def load_w(t, split=1):
                wr = wall[t * D : (t + 1) * D].rearrange("(f p) o -> p f o", p=128)
                fh = NF // split
                for k in range(split):
                    nc.sync.dma_start(
                        w_all[:, t, k * fh : (k + 1) * fh], wr[:, k * fh : (k + 1) * fh]
                    )

            load_w(0, split=2)

            xTr = xT.rearrange("(f p) t -> p f t", p=128)
            cTr = cT.rearrange("(f p) t -> p f t", p=128)

            ENGS = [nc.scalar, nc.vector]
            ectr = [0]

            def next_eng():
                e = ENGS[ectr[0] % len(ENGS)]
                ectr[0] += 1
                return e

            def emit_exp(eng, et_ap, psc_ap):
                if eng is nc.scalar:
                    eng.activation(et_ap, psc_ap, EXP)
                else:
                    eng.tensor_scalar(
                        et_ap.bitcast(I16), psc_ap, A_EXP, B_EXP, MULT, ADD
                    )

            def emit_copy(eng, dst, src):
                if eng is nc.scalar:
                    eng.copy(dst, src)
                else:
                    eng.tensor_copy(dst, src)

            def emit_scale(eng, dst, src, scale_ap):
                if eng is nc.scalar:
                    eng.mul(dst, src, scale_ap)
                else:
                    eng.tensor_scalar(dst, src, scale_ap, None, MULT)

            qT = [
                qkpool.tile([128, s], BF16, tag="qT0", name="q_o0"),
                qkpool.tile([128, s], BF16, tag="qT1", name="q_o1"),
            ]
            kT = [
                qkpool.tile([128, s], BF16, tag="kT0", name="k_o0"),
                qkpool.tile([128, s], BF16, tag="kT1", name="k_o1"),
            ]
            vab = [None] * nj

            # ---- projection phase (128x128 mode) ----
            with tc.tile_pool(name="pj", bufs=2, space="PSUM") as pj:

                def load_tok(src_r, i, nm):
                    # two half-loads split along TOKENS (each 512-token proj
                    # chunk accumulates over all 8 f, so an f-split would gate
                    # the first chunk on the full tile)
                    t = spool.tile([128, NF, TOK], BF16, tag="st", name=nm)
                    half = TOK // 2
                    tr = src_r[:, :, i * TOK : (i + 1) * TOK]
                    nc.sync.dma_start(t[:, :, :half], tr[:, :, :half])
                    nc.sync.dma_start(t[:, :, half:], tr[:, :, half:])
                    return t

                def proj_chunk(w_sb, tok, o, dst_sl):
                    pq = pj.tile([128, TOK], F32, tag="pp", name="pq")
                    for w0 in range(0, TOK, 512):
                        for f in range(NF):
                            nc.tensor.matmul(
                                pq[:, w0 : w0 + 512],
                                w_sb[:, f, o * 128 : (o + 1) * 128],
                                tok[:, f, w0 : w0 + 512],
                                start=(f == 0),
                                stop=(f == NF - 1),
                            )
                    emit_copy(next_eng(), dst_sl, pq[:])

                xt, ct = [], []
                for ib in range(ntt):
                    xt.append(load_tok(xTr, ib, f"xt{ib}"))
                    if ib == 0:
                        load_w(1)
                    ct.append(load_tok(cTr, ib, f"ct{ib}"))
                    if ib == 0:
                        load_w(2)
                    sl = slice(ib * TOK, (ib + 1) * TOK)
                    for o in range(2):
                        proj_chunk(w_all[:, 0], xt[ib], o, qT[o][:, sl])
                    for o in range(2):
                        proj_chunk(w_all[:, 1], ct[ib], o, kT[o][:, sl])

                tpj = TOK // 128
                for jc in range(nj):
                    # v[j, o] = sum_f cT[f,j] * WvT[f,o]
                    pvv = pj.tile([128, TOK], F32, tag="pp", name="pvv")
                    for f in range(NF):
                        nc.tensor.matmul(
                            pvv[:, :DH],
                            ct[jc // tpj][:, f, (jc % tpj) * 128 : (jc % tpj + 1) * 128],
                            w_all[:, 2, f, :],
                            start=(f == 0),
                            stop=(f == NF - 1),
                        )
                    va = vpool.tile([128, HPC, HD + 1], BF16, tag="vab", name="va")
                    emit_copy(
                        next_eng(),
                        va[:, :, :HD],
                        pvv[:, :DH].rearrange("p (h c) -> p h c", c=HD),
                    )
                    nc.gpsimd.memset(va[:, :, HD], 1.0)
                    vab[jc] = va

            # ---- attention phase ----
            # sub-block = (head pair hp, i chunk ic). Scores+exp of sub-block n
            # are emitted with the AV groups of sub-block n-1 spliced in (one
            # 16-matmul group per 2 score pairs).
            with (
                tc.tile_pool(name="ps", bufs=3, space="PSUM") as ps,
                tc.tile_pool(name="pv", bufs=2, space="PSUM") as pv,
            ):

                def av_group(hp, ic, ets, h01, ib):
                    h = hp * 2 + h01
                    ppv = pv.tile([128, HD + 1], F32, tag="pv", name="ppv")
                    for jt in range(nj):
                        nc.tensor.matmul(
                            ppv[:],
                            ets[jt][:, h01, ib * 128 : (ib + 1) * 128],
                            vab[jt][:, h, :],
                            start=(jt == 0),
                            stop=(jt == nj - 1),
                        )
                    # finalize: per-partition denom at free col HD
                    rd = fpool.tile([128, 1], F32, tag="rd", name="rd")
                    nc.vector.reciprocal(rd[:], ppv[:, HD : HD + 1])
                    ob = obs[h01]
                    emit_scale(
                        next_eng(),
                        ob[:, ib * HD : (ib + 1) * HD],
                        ppv[:, :HD],
                        rd[:],
                    )
                    if ib == nib - 1:
                        nc.sync.dma_start(
                            out[ic * IC : (ic + 1) * IC, h * HD : (h + 1) * HD]
                            .rearrange("(b p) c -> p b c", p=128),
                            ob[:].rearrange("p (b c) -> p b c", c=HD),
                        )

                subs = [(hp, ic) for hp in range(2) for ic in range(nic)]
                pend = None  # (hp, ic, ets) awaiting AV emission
                obs = None
                # exp engine per score pair: ScalarE amortizes per-op overhead
                # better, so it takes 9 of 16; DVE also does recips + half the
                # normalize muls.
                act_jts = {0, 2, 4, 6, 7, 9, 11, 13, 15}

                for hp, ic in subs:
                    ets = [None] * nj
                    if pend is not None:
                        pgroups = [(h01, ib) for h01 in range(2) for ib in range(nib)]
                        obs = [
                            opool.tile([128, nib * HD], F32, tag="ob", name="ob0"),
                            opool.tile([128, nib * HD], F32, tag="ob", name="ob1"),
                        ]
                    for jt in range(nj):
                        # both heads of the pair concurrently on row-tiles
                        # T0/T8, into the two banks of one PSUM tile
                        psc = ps.tile([128, 2, IC], F32, tag="sc", name="psc")
                        for h01 in range(2):
                            pb = h01 * 64
                            nc.tensor.matmul(
                                psc[:, h01],
                                kT[hp][pb : pb + 64, jt * 128 : (jt + 1) * 128],
                                qT[hp][pb : pb + 64, ic * IC : (ic + 1) * IC],
                                start=True,
                                stop=True,
                            )
                        et = epool.tile([128, 2, IC], BF16, tag="et", name="et")
                        eng = nc.scalar if jt in act_jts else nc.vector
                        emit_exp(eng, et[:], psc[:])
                        ets[jt] = et
                        if pend is not None and jt % 2 == 1:
                            av_group(pend[0], pend[1], pend[2], *pgroups[jt // 2])
                    pend = (hp, ic, ets)

                # drain: AV of the last sub-block
                obs = [
                    opool.tile([128, nib * HD], F32, tag="ob", name="ob0"),
                    opool.tile([128, nib * HD], F32, tag="ob", name="ob1"),
                ]
                for h01 in range(2):
                    for ib in range(nib):
                        av_group(pend[0], pend[1], pend[2], h01, ib)

    if split_waits:
        _split_excess_waits(nc)
    return nc


def make_in_maps(x, context, Wq, Wkv, s=S):
    """Host-side shard + layout prep. Core c -> (batch c//HG, head group c%HG)."""
    x = np.asarray(x, dtype=np.float32)
    context = np.asarray(context, dtype=np.float32)
    Wq = np.asarray(Wq, dtype=np.float32)
    Wkv = np.asarray(Wkv, dtype=np.float32)
    scale = np.float32(HD**-0.5)
    bf16 = ml_dtypes.bfloat16
    xTb = [np.ascontiguousarray(x[b].T).astype(bf16) for b in range(B)]
    cTb = [np.ascontiguousarray(context[b].T).astype(bf16) for b in range(B)]
    in_maps = []
    for core in range(N_CORES):
        b, hg = core // HG, core % HG
        sl = slice(hg * DH, (hg + 1) * DH)
        in_maps.append(
            {
                "xT": xTb[b],
                "cT": cTb[b],
                "wall": np.ascontiguousarray(
                    np.concatenate(
                        [
                            Wq[sl].T * scale,
                            Wkv[sl].T,
                            Wkv[D + hg * DH : D + (hg + 1) * DH].T,
                        ],
                        axis=0,
                    )
                ).astype(bf16),
            }
        )
    return in_maps


def gather_out(results, s=S):
    full = np.empty((B, s, D), dtype=np.float32)
    for core in range(N_CORES):
        b, hg = core // HG, core % HG
        full[b, :, hg * DH : (hg + 1) * DH] = results[core]["out"]
    return full


def kernel(x, context, Wq, Wkv):
    nc = build_nc(S)
    in_maps = make_in_maps(x, context, Wq, Wkv, S)
    res = run_bass_kernel_spmd(nc, in_maps, list(range(N_CORES)))
    return gather_out(res.results, S)
